# revision 19
# baseline (speedup 1.0000x reference)
"""Distributed Trainium2 kernel for the additive-attention alignment predictor.

Math: score[b,t,u] = sum_h w_h * tanh(ep[b,t,h] + dp[b,u,h]);  out = softmax_u(score)
  where ep = enc @ W_enc (bias folded into dp), dp = dec @ W_dec + b_enc + b_dec.
  (b_score dropped: softmax is shift-invariant; pure-x terms likewise dropped.)

tanh(z) on |z|<=6 is replaced by the separable expansion
  tanh(z) ~= c1*z + c3*z^3 + cA sin(fA z) + cB sin(fB z) + cB2 sin(2 fB z)
(half-angle base evals for fA, fB; one double-angle step for 2fB).
sin(w(x+y)) = sin cos + cos sin and the cubic expand into 9 rank-1 products,
so the whole [T,U,H] contraction becomes TensorEngine matmuls.  Sin planes are
stored as sin/2^g; the 2^g factors fold into per-partition coefficient vectors
(wq tile) that scale the y-side planes via broadcast_to.

Engine split: ACT = dp-bias adds (Identity), 4 Sin evals (fB first: it owns
the ladder), shsqA square, softmax Exp (+accum row-sum); DVE = ep casts and
the sin/cos combine + B-plane chain; GpSimd = the slack-tolerant poly B-plane
chain.  Inputs are partition-major per-k-half descriptors over both HWDGE
queues.  Dummy ones-matmuls at kernel start keep the PE HAM clock warm
through the input DMA wait.

Sharding: data-parallel over (B, T/2): core c handles batch c//2, t-half c%2.
No cross-core communication.
"""

import math

import numpy as np
import ml_dtypes

import concourse.bass as bass
import concourse.tile as tile
from concourse import bacc, mybir
from concourse.bass_utils import run_bass_kernel_spmd

# Problem shapes (hardcoded per spec)
B, T, U = 4, 800, 150
D, H = 512, 256
NCORES = 8
TPC = T * B // NCORES  # 400 t-rows per core
P = 128
KT = D // P
HT = H // P
W550 = TPC + U  # 550
TB_W = 100
TBLK = [(i * TB_W, TB_W) for i in range(TPC // TB_W)]
N_WARMUP = 20

# Fitted expansion (config D): tanh(z) ~= C1 z + C3 z^3 + sum cf sin(w z)
FA, FB = 0.88, 1.215
FREQS = [FA, FB, 2 * FB]
C1, C3 = 0.49382319, -0.01153056
CF = [-0.08788495, 0.32848088, 0.06769629]
GENS = [0, 1, 2]  # stored sin plane is sin(w z)/2^g (fA direct)

F32 = mybir.dt.float32
BF16 = mybir.dt.bfloat16
AF = mybir.ActivationFunctionType
ALU = mybir.AluOpType

# wq columns (per m): 0..2 freq coefs cf*2^g*w, 3: 3c3*w, 4: c1*w, 5: c3*w, 6: -2*cfA*w
NSLOT = 7


def _build_graph():
    nc = bacc.Bacc()
    # partition-major inputs: [P, k-major free] so DMA runs are contiguous
    enc_x = nc.declare_dram_parameter("enc_pm", [P, KT * TPC], BF16, isOutput=False)
    dec_x = nc.declare_dram_parameter("dec_pm", [P, KT * U], BF16, isOutput=False)
    we_x = nc.declare_dram_parameter("we_pm", [P, KT * H], BF16, isOutput=False)
    wd_x = nc.declare_dram_parameter("wd_pm", [P, KT * H], BF16, isOutput=False)
    wq_x = nc.declare_dram_parameter("wq", [P, NSLOT * HT], BF16, isOutput=False)
    bias_x = nc.declare_dram_parameter("bias2", [P, 3 * HT], F32, isOutput=False)
    out_x = nc.declare_dram_parameter("out", [TPC, U], F32, isOutput=True)

    with tile.TileContext(nc) as tc:
        with (
            tc.tile_pool(name="const", bufs=1) as const,
            tc.tile_pool(name="soft", bufs=1) as soft,
            tc.tile_pool(name="ppsum", bufs=1, space="PSUM") as ppsum,
            tc.tile_pool(name="spsum", bufs=1, space="PSUM") as spsum,
        ):
            # ---- input DMAs first: per-k-half descriptors, weights on sync
            enc_sb = const.tile([P, KT, TPC], BF16)
            dec_sb = const.tile([P, KT, U], BF16)
            we_sb = const.tile([P, KT, H], BF16)
            wd_sb = const.tile([P, KT, H], BF16)
            wq_sb = const.tile([P, NSLOT, HT], BF16)
            bias_sb = const.tile([P, 3 * HT], F32)
            EH = KT * TPC // 2
            WH = KT * H // 2
            nc.sync.dma_start(out=we_sb[:, 0:2, :], in_=we_x[:, 0:WH])
            nc.scalar.dma_start(out=enc_sb[:, 0:2, :], in_=enc_x[:, 0:EH])
            nc.sync.dma_start(out=wd_sb[:, 0:2, :], in_=wd_x[:, 0:WH])
            nc.scalar.dma_start(out=dec_sb, in_=dec_x[:, :])
            nc.sync.dma_start(out=we_sb[:, 2:4, :], in_=we_x[:, WH:])
            nc.gpsimd.dma_start(out=wq_sb, in_=wq_x[:])
            nc.scalar.dma_start(out=enc_sb[:, 2:4, :], in_=enc_x[:, EH:])
            nc.sync.dma_start(out=wd_sb[:, 2:4, :], in_=wd_x[:, WH:])
            nc.scalar.dma_start(out=bias_sb, in_=bias_x[:])

            # ---- constants
            ones_a = const.tile([P, P], BF16)
            nc.vector.memset(ones_a, 1.0)
            halfpi = const.tile([P, 1], F32)
            nc.vector.memset(halfpi, math.pi / 2)
            tldummy = const.tile([P, 1], F32)
            nc.scalar.activation(out=tldummy, in_=halfpi, func=AF.Sin, scale=1.0)

            # ---- PE warm-up through the DMA wait
            ps_ep = [ppsum.tile([P, TPC], F32, name=f"ps_ep{m}") for m in range(HT)]
            ps_dp = [ppsum.tile([P, U], F32, name=f"ps_dp{m}") for m in range(HT)]
            for _ in range(N_WARMUP):
                nc.tensor.matmul(
                    ps_dp[0][:, 0:P], lhsT=ones_a, rhs=ones_a,
                    start=True, stop=True, skip_group_check=True,
                )

            # ---- projections (k-halves as the DMAs land)
            for k in range(KT):
                for m in range(HT):
                    nc.tensor.matmul(
                        ps_ep[m],
                        lhsT=we_sb[:, k, m * P:(m + 1) * P],
                        rhs=enc_sb[:, k, :],
                        start=(k == 0), stop=(k == KT - 1),
                    )
                for m in range(HT):
                    nc.tensor.matmul(
                        ps_dp[m],
                        lhsT=wd_sb[:, k, m * P:(m + 1) * P],
                        rhs=dec_sb[:, k, :],
                        start=(k == 0), stop=(k == KT - 1),
                    )

            # ---- epdp [P, m, 550]: x 0:400 (DVE casts), y 400:550 (ACT adds)
            epdp = const.tile([P, HT, W550], BF16)
            for m in range(HT):
                nc.scalar.activation(
                    out=epdp[:, m, TPC:W550], in_=ps_dp[m], func=AF.Identity,
                    scale=1.0, bias=bias_sb[:, m:m + 1],
                )
                nc.vector.tensor_copy(epdp[:, m, 0:TPC], ps_ep[m])
            ep_y = epdp[:, :, TPC:W550]

            # ---- ACT: Sin evals, fB first (it owns the ladder)
            shB = const.tile([P, HT, W550], BF16)
            chB = const.tile([P, HT, W550], BF16)
            shA = const.tile([P, HT, W550], BF16)
            sA = const.tile([P, HT, W550], BF16)    # sin(fA), direct
            nc.scalar.activation(out=shB, in_=epdp, func=AF.Sin, scale=float(FB / 2))
            nc.scalar.activation(out=chB, in_=epdp, func=AF.Sin, scale=float(FB / 2), bias=halfpi[:, :])
            nc.scalar.activation(out=shA, in_=epdp, func=AF.Sin, scale=float(FA / 2))
            nc.scalar.activation(out=sA, in_=epdp, func=AF.Sin, scale=float(FA))

            cA = const.tile([P, HT, W550], BF16)
            sB = const.tile([P, HT, W550], BF16)    # sin(fB)/2
            cB = const.tile([P, HT, W550], BF16)
            s2B = const.tile([P, HT, W550], BF16)   # sin(2fB)/4
            c2B = const.tile([P, HT, W550], BF16)
            shsqA = const.tile([P, HT, W550], BF16)
            shsqB = const.tile([P, HT, W550], BF16)
            tB = const.tile([P, HT, W550], BF16)    # sB^2
            sq = const.tile([P, HT, W550], BF16)    # epdp^2

            b_s = [const.tile([P, 2, HT, U], BF16, name=f"b_f{i}") for i in range(3)]

            def b_freq(i, s_t, c_t, eng=nc.vector):
                wv = wq_sb[:, i, :].broadcast_to([P, HT, U])
                eng.tensor_tensor(out=b_s[i][:, 0], in0=s_t[:, :, TPC:W550], in1=wv, op=ALU.mult)
                eng.tensor_tensor(out=b_s[i][:, 1], in0=c_t[:, :, TPC:W550], in1=wv, op=ALU.mult)

            # ---- DVE: critical fB ladder chain first (priority order)
            nc.vector.tensor_tensor(out=shsqB, in0=shB, in1=shB, op=ALU.mult)
            nc.vector.tensor_tensor(out=sB, in0=shB, in1=chB, op=ALU.mult)
            nc.vector.tensor_scalar(out=cB, in0=shsqB, scalar1=-2.0, scalar2=1.0, op0=ALU.mult, op1=ALU.add)
            b_freq(1, sB, cB)
            nc.vector.tensor_tensor(out=tB, in0=sB, in1=sB, op=ALU.mult)
            nc.vector.tensor_tensor(out=s2B, in0=sB, in1=cB, op=ALU.mult)
            nc.vector.tensor_scalar(out=c2B, in0=tB, scalar1=-8.0, scalar2=1.0, op0=ALU.mult, op1=ALU.add)
            b_freq(2, s2B, c2B)

            # fA chain entirely on ACT's idle tail: cA x-part for the lhsT, and
            # the y-side B planes fused via per-partition scale/bias
            # (cos_y * w cf = -2 w cf * shsqA_y + w cf; sin_y * w cf directly)
            nc.scalar.activation(out=shsqA, in_=shA, func=AF.Square, scale=1.0)
            nc.scalar.activation(out=cA[:, :, 0:TPC], in_=shsqA[:, :, 0:TPC],
                                 func=AF.Identity, scale=-2.0, bias=1.0)
            for m in range(HT):
                nc.scalar.activation(out=b_s[0][:, 0, m], in_=sA[:, m, TPC:W550],
                                     func=AF.Identity, scale=bias_sb[:, 2 + m:3 + m])
                nc.scalar.activation(out=b_s[0][:, 1, m], in_=shsqA[:, m, TPC:W550],
                                     func=AF.Identity, scale=bias_sb[:, 4 + m:5 + m],
                                     bias=bias_sb[:, 2 + m:3 + m])

            # epdp^2 fills the DVE idle gap before shB lands
            nc.vector.tensor_tensor(out=sq, in0=epdp, in1=epdp, op=ALU.mult)
            sq_y = sq[:, :, TPC:W550]

            # ---- poly B-plane chain (DVE, low priority: fills idle gaps)
            wv3 = wq_sb[:, 3, :].broadcast_to([P, HT, U])
            b_x2 = const.tile([P, HT, U], BF16)   # (3c3 w) y
            nc.gpsimd.tensor_tensor(out=b_x2, in0=ep_y, in1=wv3, op=ALU.mult)
            b_l1 = const.tile([P, HT, U], BF16)   # (c1 w) y
            nc.gpsimd.tensor_tensor(out=b_l1, in0=ep_y, in1=wq_sb[:, 4, :].broadcast_to([P, HT, U]), op=ALU.mult)
            v3 = const.tile([P, HT, U], BF16)     # y^3
            nc.gpsimd.tensor_tensor(out=v3, in0=ep_y, in1=sq_y, op=ALU.mult)
            b_x = const.tile([P, HT, U], BF16)    # (y^2 + c1/(3c3)) (3c3 w)
            nc.vector.scalar_tensor_tensor(
                out=b_x, in0=sq_y, scalar=float(C1 / (3 * C3)), in1=wv3,
                op0=ALU.add, op1=ALU.mult,
            )
            b_l3 = const.tile([P, HT, U], BF16)   # (c3 w) y^3
            nc.gpsimd.tensor_tensor(out=b_l3, in0=v3, in1=wq_sb[:, 5, :].broadcast_to([P, HT, U]), op=ALU.mult)
            b_lin = const.tile([P, HT, U], BF16)
            nc.gpsimd.tensor_tensor(out=b_lin, in0=b_l1, in1=b_l3, op=ALU.add)
            b_lin2 = const.tile([P, U], BF16)     # m-sum for the ones-lhsT pair
            nc.gpsimd.tensor_tensor(out=b_lin2, in0=b_lin[:, 0], in1=b_lin[:, 1], op=ALU.add)

            # ---- content matmuls: one private psum bank per t-block
            sp = [spsum.tile([P, 512], F32, name=f"sp{tb}") for tb in range(len(TBLK))]
            outt_all = soft.tile([P, len(TBLK), U], F32)
            out_v = out_x[:].rearrange("(tb p) u -> p tb u", p=TB_W)
            mm_i = [0] * len(TBLK)
            N_MM = 17  # per t-block: 8 m-paired entries + ones pair

            def emit_phase(entries, final=False):
                for tb, (t0, pn) in enumerate(TBLK):
                    sl = slice(t0, t0 + pn)
                    for ent in entries:
                        if ent[0] == "ones":
                            nc.tensor.matmul(
                                sp[tb][:pn, 0:U], lhsT=ones_a[:, :pn], rhs=ent[1],
                                start=(mm_i[tb] == 0), stop=(mm_i[tb] == N_MM - 1),
                            )
                            mm_i[tb] += 1
                        else:
                            a_fn, b_fn = ent
                            for m in range(HT):
                                nc.tensor.matmul(
                                    sp[tb][:pn, 0:U], lhsT=a_fn(m, sl), rhs=b_fn(m),
                                    start=(mm_i[tb] == 0), stop=(mm_i[tb] == N_MM - 1),
                                )
                                mm_i[tb] += 1
                    if final:
                        expt = soft.tile([P, U], F32, name=f"expt{tb}", bufs=4)
                        ssum = soft.tile([P, 1], F32, name=f"ssum{tb}", bufs=4)
                        nc.scalar.activation(
                            out=expt[:pn], in_=sp[tb][:pn, 0:U], func=AF.Exp, scale=1.0,
                        )
                        nc.vector.tensor_reduce(
                            out=ssum[:pn], in_=expt[:pn], axis=mybir.AxisListType.X, op=ALU.add,
                        )
                        nc.vector.reciprocal(out=ssum[:pn], in_=ssum[:pn])
                        nc.vector.tensor_scalar_mul(
                            out=outt_all[:pn, tb, :], in0=expt[:pn], scalar1=ssum[:pn]
                        )
                if final:
                    nc.sync.dma_start(out=out_v, in_=outt_all[0:TB_W])

            def freq_entries(i, s_t, c_t):
                return [
                    (lambda m, s, t=s_t: t[:, m, s], lambda m, i=i: b_s[i][:, 1, m]),
                    (lambda m, s, t=c_t: t[:, m, s], lambda m, i=i: b_s[i][:, 0, m]),
                ]

            emit_phase(freq_entries(1, sB, cB))            # fB
            emit_phase([                                    # poly
                (lambda m, s: epdp[:, m, s], lambda m: b_x[:, m]),
                (lambda m, s: sq[:, m, s], lambda m: b_x2[:, m]),
                ("ones", b_lin2),
            ])
            emit_phase(freq_entries(2, s2B, c2B))          # 2fB
            emit_phase(freq_entries(0, sA, cA), final=True)  # fA + softmax

    nc.finalize()
    return nc


_NC_CACHE = None


def kernel(**inputs: np.ndarray) -> np.ndarray:
    global _NC_CACHE
    bfd = ml_dtypes.bfloat16
    enc = np.asarray(inputs["encoder_out"], dtype=np.float32)
    dec = np.asarray(inputs["decoder_out"], dtype=np.float32)
    w_enc = np.asarray(inputs["W_enc"], np.float32)
    b_enc = np.asarray(inputs["b_enc"], dtype=np.float32)
    w_dec = np.asarray(inputs["W_dec"], np.float32)
    b_dec = np.asarray(inputs["b_dec"], dtype=np.float32)
    w_score = np.asarray(inputs["w_score"], dtype=np.float32)
    # b_score dropped: softmax(x + c) == softmax(x)

    def part_major(a2d):
        # [D, X] -> [P, KT*X]: partition p holds rows {k*P+p} k-major, contiguous
        X = a2d.shape[1]
        return np.ascontiguousarray(
            a2d.reshape(KT, P, X).transpose(1, 0, 2).reshape(P, KT * X).astype(bfd)
        )

    we_pm = part_major(w_enc)
    wd_pm = part_major(w_dec)
    dec_pm = [part_major(dec[b].T) for b in range(B)]
    bias2 = np.empty((P, 3 * HT), dtype=np.float32)
    bias2[:, 0:HT] = (b_enc + b_dec).reshape(HT, P).T
    wm0 = w_score.reshape(HT, P).T
    bias2[:, HT:2 * HT] = CF[0] * wm0
    bias2[:, 2 * HT:3 * HT] = -2 * CF[0] * wm0
    bias2 = np.ascontiguousarray(bias2)

    # wq[p, slot, m]: 0..2 freq coefs cf*2^g*w, 3: 3c3*w, 4: c1*w, 5: c3*w
    wm = w_score.reshape(HT, P).T  # [P, m]
    wq = np.empty((P, NSLOT, HT), dtype=np.float32)
    for i in range(3):
        wq[:, i] = CF[i] * (2 ** GENS[i]) * wm
    wq[:, 3] = 3 * C3 * wm
    wq[:, 4] = C1 * wm
    wq[:, 5] = C3 * wm
    wq[:, 6] = -2 * CF[0] * wm
    wq = np.ascontiguousarray(wq.reshape(P, NSLOT * HT).astype(bfd))

    in_maps = []
    for c in range(NCORES):
        b = c // (NCORES // B)
        t0 = (c % (NCORES // B)) * TPC
        in_maps.append(
            {
                "enc_pm": part_major(enc[b, t0:t0 + TPC, :].T),
                "dec_pm": dec_pm[b],
                "we_pm": we_pm,
                "wd_pm": wd_pm,
                "wq": wq,
                "bias2": bias2,
            }
        )

    if _NC_CACHE is None:
        _NC_CACHE = _build_graph()
    res = run_bass_kernel_spmd(_NC_CACHE, in_maps, core_ids=list(range(NCORES)))

    out = np.empty((B, T, U), dtype=np.float32)
    for c in range(NCORES):
        b = c // (NCORES // B)
        t0 = (c % (NCORES // B)) * TPC
        out[b, t0:t0 + TPC, :] = res.results[c]["out"]
    return out


# revision 20
# speedup vs baseline: 1.0214x; 1.0214x over previous
"""Distributed Trainium2 kernel for the additive-attention alignment predictor.

Math: score[b,t,u] = sum_h w_h * tanh(ep[b,t,h] + dp[b,u,h]);  out = softmax_u(score)
  where ep = enc @ W_enc (bias folded into dp), dp = dec @ W_dec + b_enc + b_dec.
  (b_score dropped: softmax is shift-invariant; pure-x terms likewise dropped.)

tanh(z) on |z|<=6 is replaced by the separable expansion
  tanh(z) ~= c1*z + c3*z^3 + cA sin(fA z) + cB sin(fB z) + cB2 sin(2 fB z)
(half-angle base evals for fA, fB; one double-angle step for 2fB).
sin(w(x+y)) = sin cos + cos sin and the cubic expand into 9 rank-1 products,
so the whole [T,U,H] contraction becomes TensorEngine matmuls.  Sin planes are
stored as sin/2^g; the 2^g factors fold into per-partition coefficient vectors
(wq tile) that scale the y-side planes via broadcast_to.

Engine split: ACT = dp-bias adds (Identity), 4 Sin evals (fB first: it owns
the ladder), shsqA square, softmax Exp (+accum row-sum); DVE = ep casts and
the sin/cos combine + B-plane chain; GpSimd = the slack-tolerant poly B-plane
chain.  Inputs are partition-major per-k-half descriptors over both HWDGE
queues.  Dummy ones-matmuls at kernel start keep the PE HAM clock warm
through the input DMA wait.

Sharding: data-parallel over (B, T/2): core c handles batch c//2, t-half c%2.
No cross-core communication.
"""

import math

import numpy as np
import ml_dtypes

import concourse.bass as bass
import concourse.tile as tile
from concourse import bacc, mybir
from concourse.bass_utils import run_bass_kernel_spmd

# Problem shapes (hardcoded per spec)
B, T, U = 4, 800, 150
D, H = 512, 256
NCORES = 8
TPC = T * B // NCORES  # 400 t-rows per core
P = 128
KT = D // P
HT = H // P
W550 = TPC + U  # 550
TB_W = 100
TBLK = [(i * TB_W, TB_W) for i in range(TPC // TB_W)]
N_WARMUP = 20

# Fitted expansion (config D): tanh(z) ~= C1 z + C3 z^3 + sum cf sin(w z)
FA, FB = 0.88, 1.215
FREQS = [FA, FB, 2 * FB]
C1, C3 = 0.49382319, -0.01153056
CF = [-0.08788495, 0.32848088, 0.06769629]
GENS = [0, 1, 2]  # stored sin plane is sin(w z)/2^g (fA direct)

F32 = mybir.dt.float32
BF16 = mybir.dt.bfloat16
AF = mybir.ActivationFunctionType
ALU = mybir.AluOpType

# wq columns (per m): 0..2 freq coefs cf*2^g*w, 3: 3c3*w, 4: c1*w, 5: c3*w
NSLOT = 6


def _build_graph():
    nc = bacc.Bacc()
    # partition-major inputs: [P, k-major free] so DMA runs are contiguous
    enc_x = nc.declare_dram_parameter("enc_pm", [P, KT * TPC], BF16, isOutput=False)
    dec_x = nc.declare_dram_parameter("dec_pm", [P, KT * U], BF16, isOutput=False)
    we_x = nc.declare_dram_parameter("we_pm", [P, KT * H], BF16, isOutput=False)
    wd_x = nc.declare_dram_parameter("wd_pm", [P, KT * H], BF16, isOutput=False)
    wq_x = nc.declare_dram_parameter("wq", [P, NSLOT * HT], BF16, isOutput=False)
    bias_x = nc.declare_dram_parameter("bias2", [P, HT], F32, isOutput=False)
    out_x = nc.declare_dram_parameter("out", [TPC, U], F32, isOutput=True)

    with tile.TileContext(nc) as tc:
        with (
            tc.tile_pool(name="const", bufs=1) as const,
            tc.tile_pool(name="soft", bufs=1) as soft,
            tc.tile_pool(name="ppsum", bufs=1, space="PSUM") as ppsum,
            tc.tile_pool(name="spsum", bufs=1, space="PSUM") as spsum,
        ):
            # ---- input DMAs first: per-k-half descriptors, weights on sync
            enc_sb = const.tile([P, KT, TPC], BF16)
            dec_sb = const.tile([P, KT, U], BF16)
            we_sb = const.tile([P, KT, H], BF16)
            wd_sb = const.tile([P, KT, H], BF16)
            wq_sb = const.tile([P, NSLOT, HT], BF16)
            bias_sb = const.tile([P, HT], F32)
            EH = KT * TPC // 2
            WH = KT * H // 2
            nc.sync.dma_start(out=we_sb[:, 0:2, :], in_=we_x[:, 0:WH])
            nc.scalar.dma_start(out=enc_sb[:, 0:2, :], in_=enc_x[:, 0:EH])
            nc.sync.dma_start(out=wd_sb[:, 0:2, :], in_=wd_x[:, 0:WH])
            nc.scalar.dma_start(out=dec_sb, in_=dec_x[:, :])
            nc.sync.dma_start(out=we_sb[:, 2:4, :], in_=we_x[:, WH:])
            nc.gpsimd.dma_start(out=wq_sb, in_=wq_x[:])
            nc.scalar.dma_start(out=bias_sb, in_=bias_x[:])
            nc.sync.dma_start(out=wd_sb[:, 2:4, :], in_=wd_x[:, WH:])
            nc.scalar.dma_start(out=enc_sb[:, 2:4, :], in_=enc_x[:, EH:])

            # ---- constants
            ones_a = const.tile([P, P], BF16)
            nc.vector.memset(ones_a, 1.0)
            halfpi = const.tile([P, 1], F32)
            nc.vector.memset(halfpi, math.pi / 2)
            tldummy = const.tile([P, 1], F32)
            nc.scalar.activation(out=tldummy, in_=halfpi, func=AF.Sin, scale=1.0)

            # ---- PE warm-up through the DMA wait
            ps_ep = [ppsum.tile([P, TPC], F32, name=f"ps_ep{m}") for m in range(HT)]
            ps_dp = [ppsum.tile([P, U], F32, name=f"ps_dp{m}") for m in range(HT)]
            for _ in range(N_WARMUP):
                nc.tensor.matmul(
                    ps_dp[0][:, 0:P], lhsT=ones_a, rhs=ones_a,
                    start=True, stop=True, skip_group_check=True,
                )

            # ---- projections (k-halves as the DMAs land)
            for k in range(KT):
                for m in range(HT):
                    nc.tensor.matmul(
                        ps_ep[m],
                        lhsT=we_sb[:, k, m * P:(m + 1) * P],
                        rhs=enc_sb[:, k, :],
                        start=(k == 0), stop=(k == KT - 1),
                    )
                for m in range(HT):
                    nc.tensor.matmul(
                        ps_dp[m],
                        lhsT=wd_sb[:, k, m * P:(m + 1) * P],
                        rhs=dec_sb[:, k, :],
                        start=(k == 0), stop=(k == KT - 1),
                    )

            # ---- epdp [P, m, 550]: x 0:400 (DVE casts), y 400:550 (ACT adds)
            epdp = const.tile([P, HT, W550], BF16)
            for m in range(HT):
                nc.scalar.activation(
                    out=epdp[:, m, TPC:W550], in_=ps_dp[m], func=AF.Identity,
                    scale=1.0, bias=bias_sb[:, m:m + 1],
                )
                nc.vector.tensor_copy(epdp[:, m, 0:TPC], ps_ep[m])
            ep_y = epdp[:, :, TPC:W550]

            # ---- ACT: Sin evals, fB first (it owns the ladder)
            shB = const.tile([P, HT, W550], BF16)
            chB = const.tile([P, HT, W550], BF16)
            shA = const.tile([P, HT, W550], BF16)
            sA = const.tile([P, HT, W550], BF16)    # sin(fA), direct
            nc.scalar.activation(out=shB, in_=epdp, func=AF.Sin, scale=float(FB / 2))
            nc.scalar.activation(out=chB, in_=epdp, func=AF.Sin, scale=float(FB / 2), bias=halfpi[:, :])
            nc.scalar.activation(out=shA, in_=epdp, func=AF.Sin, scale=float(FA / 2))
            nc.scalar.activation(out=sA, in_=epdp, func=AF.Sin, scale=float(FA))

            cA = const.tile([P, HT, W550], BF16)
            sB = const.tile([P, HT, W550], BF16)    # sin(fB)/2
            cB = const.tile([P, HT, W550], BF16)
            s2B = const.tile([P, HT, W550], BF16)   # sin(2fB)/4
            c2B = const.tile([P, HT, W550], BF16)
            shsqA = const.tile([P, HT, W550], BF16)
            shsqB = const.tile([P, HT, W550], BF16)
            tB = const.tile([P, HT, W550], BF16)    # sB^2
            sq = const.tile([P, HT, W550], BF16)    # epdp^2

            b_s = [const.tile([P, 2, HT, U], BF16, name=f"b_f{i}") for i in range(3)]

            def b_freq(i, s_t, c_t, eng=nc.vector):
                wv = wq_sb[:, i, :].broadcast_to([P, HT, U])
                eng.tensor_tensor(out=b_s[i][:, 0], in0=s_t[:, :, TPC:W550], in1=wv, op=ALU.mult)
                eng.tensor_tensor(out=b_s[i][:, 1], in0=c_t[:, :, TPC:W550], in1=wv, op=ALU.mult)

            # ---- DVE: critical fB ladder chain first (priority order)
            nc.vector.tensor_tensor(out=shsqB, in0=shB, in1=shB, op=ALU.mult)
            nc.vector.tensor_tensor(out=sB, in0=shB, in1=chB, op=ALU.mult)
            nc.vector.tensor_scalar(out=cB, in0=shsqB, scalar1=-2.0, scalar2=1.0, op0=ALU.mult, op1=ALU.add)
            b_freq(1, sB, cB)
            nc.vector.tensor_tensor(out=tB, in0=sB, in1=sB, op=ALU.mult)
            nc.vector.tensor_tensor(out=s2B, in0=sB, in1=cB, op=ALU.mult)
            nc.vector.tensor_scalar(out=c2B, in0=tB, scalar1=-8.0, scalar2=1.0, op0=ALU.mult, op1=ALU.add)
            b_freq(2, s2B, c2B)

            # fA chain (sA came straight from ACT; shsqA square on ACT's tail)
            nc.scalar.activation(out=shsqA, in_=shA, func=AF.Square, scale=1.0)
            nc.vector.tensor_scalar(out=cA, in0=shsqA, scalar1=-2.0, scalar2=1.0, op0=ALU.mult, op1=ALU.add)
            b_freq(0, sA, cA)

            # epdp^2 fills the DVE idle gap before shB lands
            nc.vector.tensor_tensor(out=sq, in0=epdp, in1=epdp, op=ALU.mult)
            sq_y = sq[:, :, TPC:W550]

            # ---- poly B-plane chain (DVE, low priority: fills idle gaps)
            wv3 = wq_sb[:, 3, :].broadcast_to([P, HT, U])
            b_x2 = const.tile([P, HT, U], BF16)   # (3c3 w) y
            nc.gpsimd.tensor_tensor(out=b_x2, in0=ep_y, in1=wv3, op=ALU.mult)
            b_l1 = const.tile([P, HT, U], BF16)   # (c1 w) y
            nc.gpsimd.tensor_tensor(out=b_l1, in0=ep_y, in1=wq_sb[:, 4, :].broadcast_to([P, HT, U]), op=ALU.mult)
            v3 = const.tile([P, HT, U], BF16)     # y^3
            nc.gpsimd.tensor_tensor(out=v3, in0=ep_y, in1=sq_y, op=ALU.mult)
            b_x = const.tile([P, HT, U], BF16)    # (y^2 + c1/(3c3)) (3c3 w)
            nc.vector.scalar_tensor_tensor(
                out=b_x, in0=sq_y, scalar=float(C1 / (3 * C3)), in1=wv3,
                op0=ALU.add, op1=ALU.mult,
            )
            b_l3 = const.tile([P, HT, U], BF16)   # (c3 w) y^3
            nc.gpsimd.tensor_tensor(out=b_l3, in0=v3, in1=wq_sb[:, 5, :].broadcast_to([P, HT, U]), op=ALU.mult)
            b_lin = const.tile([P, HT, U], BF16)
            nc.gpsimd.tensor_tensor(out=b_lin, in0=b_l1, in1=b_l3, op=ALU.add)
            b_lin2 = const.tile([P, U], BF16)     # m-sum for the ones-lhsT pair
            nc.gpsimd.tensor_tensor(out=b_lin2, in0=b_lin[:, 0], in1=b_lin[:, 1], op=ALU.add)

            # ---- content matmuls: one private psum bank per t-block
            sp = [spsum.tile([P, 512], F32, name=f"sp{tb}") for tb in range(len(TBLK))]
            outt_all = soft.tile([P, len(TBLK), U], F32)
            out_v = out_x[:].rearrange("(tb p) u -> p tb u", p=TB_W)
            mm_i = [0] * len(TBLK)
            N_MM = 17  # per t-block: 8 m-paired entries + ones pair

            def emit_phase(entries, final=False):
                for tb, (t0, pn) in enumerate(TBLK):
                    sl = slice(t0, t0 + pn)
                    for ent in entries:
                        if ent[0] == "ones":
                            nc.tensor.matmul(
                                sp[tb][:pn, 0:U], lhsT=ones_a[:, :pn], rhs=ent[1],
                                start=(mm_i[tb] == 0), stop=(mm_i[tb] == N_MM - 1),
                            )
                            mm_i[tb] += 1
                        else:
                            a_fn, b_fn = ent
                            for m in range(HT):
                                nc.tensor.matmul(
                                    sp[tb][:pn, 0:U], lhsT=a_fn(m, sl), rhs=b_fn(m),
                                    start=(mm_i[tb] == 0), stop=(mm_i[tb] == N_MM - 1),
                                )
                                mm_i[tb] += 1
                    if final:
                        expt = soft.tile([P, U], F32, name=f"expt{tb}", bufs=4)
                        ssum = soft.tile([P, 1], F32, name=f"ssum{tb}", bufs=4)
                        nc.scalar.activation(
                            out=expt[:pn], in_=sp[tb][:pn, 0:U], func=AF.Exp,
                            scale=1.0, accum_out=ssum[:pn],
                        )
                        nc.vector.reciprocal(out=ssum[:pn], in_=ssum[:pn])
                        nc.vector.tensor_scalar_mul(
                            out=outt_all[:pn, tb, :], in0=expt[:pn], scalar1=ssum[:pn]
                        )
                if final:
                    nc.sync.dma_start(out=out_v, in_=outt_all[0:TB_W])

            def freq_entries(i, s_t, c_t):
                return [
                    (lambda m, s, t=s_t: t[:, m, s], lambda m, i=i: b_s[i][:, 1, m]),
                    (lambda m, s, t=c_t: t[:, m, s], lambda m, i=i: b_s[i][:, 0, m]),
                ]

            emit_phase(freq_entries(1, sB, cB))            # fB
            emit_phase([                                    # poly
                (lambda m, s: epdp[:, m, s], lambda m: b_x[:, m]),
                (lambda m, s: sq[:, m, s], lambda m: b_x2[:, m]),
                ("ones", b_lin2),
            ])
            emit_phase(freq_entries(2, s2B, c2B))          # 2fB
            emit_phase(freq_entries(0, sA, cA), final=True)  # fA + softmax

    nc.finalize()
    return nc


_NC_CACHE = None


def kernel(**inputs: np.ndarray) -> np.ndarray:
    global _NC_CACHE
    bfd = ml_dtypes.bfloat16
    enc = np.asarray(inputs["encoder_out"], dtype=np.float32)
    dec = np.asarray(inputs["decoder_out"], dtype=np.float32)
    w_enc = np.asarray(inputs["W_enc"], np.float32)
    b_enc = np.asarray(inputs["b_enc"], dtype=np.float32)
    w_dec = np.asarray(inputs["W_dec"], np.float32)
    b_dec = np.asarray(inputs["b_dec"], dtype=np.float32)
    w_score = np.asarray(inputs["w_score"], dtype=np.float32)
    # b_score dropped: softmax(x + c) == softmax(x)

    def part_major(a2d):
        # [D, X] -> [P, KT*X]: partition p holds rows {k*P+p} k-major, contiguous
        X = a2d.shape[1]
        return np.ascontiguousarray(
            a2d.reshape(KT, P, X).transpose(1, 0, 2).reshape(P, KT * X).astype(bfd)
        )

    we_pm = part_major(w_enc)
    wd_pm = part_major(w_dec)
    dec_pm = [part_major(dec[b].T) for b in range(B)]
    bias2 = np.ascontiguousarray((b_enc + b_dec).reshape(HT, P).T)

    # wq[p, slot, m]: 0..2 freq coefs cf*2^g*w, 3: 3c3*w, 4: c1*w, 5: c3*w
    wm = w_score.reshape(HT, P).T  # [P, m]
    wq = np.empty((P, NSLOT, HT), dtype=np.float32)
    for i in range(3):
        wq[:, i] = CF[i] * (2 ** GENS[i]) * wm
    wq[:, 3] = 3 * C3 * wm
    wq[:, 4] = C1 * wm
    wq[:, 5] = C3 * wm
    wq = np.ascontiguousarray(wq.reshape(P, NSLOT * HT).astype(bfd))

    in_maps = []
    for c in range(NCORES):
        b = c // (NCORES // B)
        t0 = (c % (NCORES // B)) * TPC
        in_maps.append(
            {
                "enc_pm": part_major(enc[b, t0:t0 + TPC, :].T),
                "dec_pm": dec_pm[b],
                "we_pm": we_pm,
                "wd_pm": wd_pm,
                "wq": wq,
                "bias2": bias2,
            }
        )

    if _NC_CACHE is None:
        _NC_CACHE = _build_graph()
    res = run_bass_kernel_spmd(_NC_CACHE, in_maps, core_ids=list(range(NCORES)))

    out = np.empty((B, T, U), dtype=np.float32)
    for c in range(NCORES):
        b = c // (NCORES // B)
        t0 = (c % (NCORES // B)) * TPC
        out[b, t0:t0 + TPC, :] = res.results[c]["out"]
    return out
